# revision 12
# baseline (speedup 1.0000x reference)
"""Trainium2 Bass kernel for the AttentionModule problem.

Cross-attention with normalized-position RoPE:
  q = Wq @ x;  k = Wk @ ctx;  v = Wv @ ctx  (per-head RoPE on q, k)
  out = Wo @ (softmax(q^T k / sqrt(512)) @ v), masked.

Sharding: 8 cores = 4 batches x 2 T-halves. Each core computes the full
module for (batch b, query half th) with all heads; host concatenates.
No collectives needed.

Layouts on device (feature-major, partition = feature):
  x_c   [512 dm, 1024 t],  ctxT [512 dc, 2048 l]  (host-transposed)
  Q/K   [a, t|l] via matmul lhsT=W^T chunks; RoPE via twin projection with
        host-rotated weights (Qr = (R'Wq) @ x) and on-device sin/cos tables.
  S_h   [l, t] = K_h^T Q_h (row-paired head pairs on the PE array)
  E     = exp(S/scale + log cmask)  (ACT, no max-subtraction: |logits|<~0.4)
  O_h   [65, t] = [V_h | 1]^T E  -> row 64 holds softmax denominators
  out   [dm, t] = Wo^T (O / s) * xmask
"""

import math
import sys
import types

sys.path.insert(0, "/opt/trn_rl_repo")

import numpy as np

import concourse.bass as bass
import concourse.tile as tile
from concourse import bacc, mybir
from concourse.bass_utils import run_bass_kernel_spmd

# Problem constants (hardcoded per spec; kernel.py must be self-contained)
D_MODEL = 512
D_CONTEXT = 512
NUM_HEADS = 8
ATTN_DIM = 512
HEAD_DIM = 64
ROPE_GAMMA = 10.0
ATTN_SCALE = math.sqrt(ATTN_DIM)
B = 4
T_FULL = 2048
L = 2048
N_CORES = 8
T = T_FULL // 2  # per-core query slice
P = 128
NAC = ATTN_DIM // P  # 4 chunks of 128 on the feature dim
NLC = L // P  # 16 l-chunks
FP32 = mybir.dt.float32
BF16 = mybir.dt.bfloat16
AF = mybir.ActivationFunctionType
ALU = mybir.AluOpType

_GRAPH_CACHE = {}


def _ensure_ntff_hook():
    """antenv.axon_hooks is absent in some images; inject it so trace=True
    can produce exec_time_ns. Harmless if tracing is never requested."""
    if "antenv.axon_hooks" in sys.modules:
        return
    try:
        mod = types.ModuleType("antenv.axon_hooks")
        mod._hook = None
        mod.set_axon_ntff_profile_hook = lambda h: setattr(mod, "_hook", h)
        mod.get_axon_ntff_profile_hook = lambda: mod._hook
        sys.modules["antenv.axon_hooks"] = mod
        from trn_agent_boot.trn_boot import _ntff_profile_via_ctypes

        mod.set_axon_ntff_profile_hook(
            _ntff_profile_via_ctypes("/opt/axon/libaxon_pjrt.so")
        )
    except Exception:
        pass


def _build_graph(use_bias: bool, use_cmask: bool, use_xmask: bool):
    nc = bacc.Bacc("TRN2", target_bir_lowering=False, debug=False, num_devices=N_CORES)

    x_d = nc.dram_tensor("x", [D_MODEL, T], FP32, kind="ExternalInput").ap()
    ctx_d = nc.dram_tensor("ctxT", [D_CONTEXT, L], FP32, kind="ExternalInput").ap()
    w_d = {
        name: nc.dram_tensor(name, [512, 512], FP32, kind="ExternalInput").ap()
        for name in ("wq", "wqr", "wk", "wkr", "wv", "wo")
    }
    rope_d = nc.dram_tensor("rope", [P, 6], FP32, kind="ExternalInput").ap()
    if use_bias:
        bias_d = nc.dram_tensor("biases", [6, 512], FP32, kind="ExternalInput").ap()
    if use_cmask:
        logcm_d = nc.dram_tensor("logcm", [P, NLC], FP32, kind="ExternalInput").ap()
    if use_xmask:
        xmask_d = nc.dram_tensor("xmaskb", [P, T], FP32, kind="ExternalInput").ap()
    out_d = nc.dram_tensor("out", [D_MODEL, T], FP32, kind="ExternalOutput").ap()

    inv_scale = 1.0 / ATTN_SCALE

    with tile.TileContext(nc) as tc:
        with (
            tc.tile_pool(name="const", bufs=1) as const,
            tc.tile_pool(name="big", bufs=1) as big,
            tc.tile_pool(name="stage", bufs=2) as stage,
            tc.tile_pool(name="tmp", bufs=2) as tmp_pool,
            tc.tile_pool(name="nrm", bufs=2) as nrm_pool,
            tc.tile_pool(name="outp", bufs=3) as out_pool,
            tc.tile_pool(name="epool", bufs=2) as e_pool,
        ):
            # ---- constants: rope params, iota, sin/cos tables ----
            rope_sb = const.tile([P, 6], FP32)
            nc.sync.dma_start(rope_sb[:], rope_d[:])
            iota_t = stage.tile([P, L], FP32, tag="stage")
            nc.gpsimd.iota(
                iota_t[:],
                [[1, L]],
                channel_multiplier=0,
                allow_small_or_imprecise_dtypes=True,
            )
            # Sin(x) on ACT needs x in [-pi, pi]: angles are in [0, ~11.6],
            # so shift by -2pi and wrap once (add_range_wrap covers
            # ang in [-pi, 5pi]), then evaluate Sin.
            cos_q = const.tile([P, T], FP32)
            sin_q = const.tile([P, T], FP32)
            cos_k = const.tile([P, L], FP32)
            sin_k = const.tile([P, L], FP32)
            ang = const.tile([P, L], FP32)
            zero_b = const.tile([P, 1], FP32)
            nc.vector.memset(zero_b[:], 0.0)
            two_pi = 2.0 * math.pi
            for table, n, s_ap, b_ap in (
                (cos_q, T, rope_sb[:, 0:1], rope_sb[:, 2:3]),
                (sin_q, T, rope_sb[:, 0:1], rope_sb[:, 1:2]),
                (cos_k, L, rope_sb[:, 3:4], rope_sb[:, 5:6]),
                (sin_k, L, rope_sb[:, 3:4], rope_sb[:, 4:5]),
            ):
                nc.vector.tensor_scalar(
                    ang[:, :n], iota_t[:, :n], s_ap, b_ap, ALU.mult, ALU.add
                )
                nc.vector.add_range_wrap(
                    ang[:, :n], ang[:, :n], shift=-two_pi, bound=math.pi, period=two_pi
                )
                nc.scalar.activation(table[:], ang[:, :n], AF.Sin, bias=zero_b[:])
            if use_cmask:
                logcm_sb = const.tile([P, NLC], FP32)
                nc.sync.dma_start(logcm_sb[:], logcm_d[:])
            if use_xmask:
                xmask_sb = const.tile([P, T], FP32)
                nc.sync.dma_start(xmask_sb[:], xmask_d[:])
            if use_bias:
                bias_st = stage.tile([6, 512], FP32, tag="bstage")
                nc.sync.dma_start(bias_st[:], bias_d[:])
                bias_bf = const.tile([1, 6, 512], BF16)
                nc.vector.tensor_copy(
                    bias_bf[:], bias_st[:].rearrange("b a -> 1 b a")
                )
                ones_row = const.tile([1, 512], BF16)
                nc.vector.memset(ones_row[:], 1.0)
                ones_col = const.tile([1, P], BF16)
                nc.vector.memset(ones_col[:], 1.0)

            # ---- load + cast inputs to bf16 ----
            x_bf = big.tile([P, NAC, T], BF16)
            for c in range(NAC):
                st = stage.tile([P, L], FP32, tag="stage")
                nc.sync.dma_start(st[:, :T], x_d[c * P : (c + 1) * P, :])
                nc.vector.tensor_copy(x_bf[:, c], st[:, :T])
            ctx_bf = big.tile([P, NAC, L], BF16)
            for c in range(NAC):
                st = stage.tile([P, L], FP32, tag="stage")
                nc.sync.dma_start(st[:], ctx_d[c * P : (c + 1) * P, :])
                nc.vector.tensor_copy(ctx_bf[:, c], st[:])
            w_bf = {}
            for name in ("wq", "wqr", "wk", "wkr", "wv", "wo"):
                st = stage.tile([P, L], FP32, tag="stage")
                stv = st[:].rearrange("p (c a) -> p c a", c=NAC)
                nc.sync.dma_start(stv, w_d[name].rearrange("(c p) a -> p c a", p=P))
                wt = big.tile([P, NAC, 512], BF16, tag=f"w_{name}")
                nc.vector.tensor_copy(wt[:], stv)
                w_bf[name] = wt

            q_rope = big.tile([P, NAC, T], BF16)
            k_rope = big.tile([P, NAC, L], BF16)
            v1 = big.tile([P, NLC, NUM_HEADS, HEAD_DIM + 1], BF16)
            nc.vector.memset(v1[:, :, :, HEAD_DIM : HEAD_DIM + 1], 1.0)
            o_norm = big.tile([P, NAC, T], BF16)

            # ---- phase 3: projections + rope (psum pool scoped) ----
            with tc.tile_pool(name="ps3", bufs=6, space="PSUM") as ps3:

                def proj_pair(wn, wrn, rhs_tile, n_groups, out_tile, cos_t, sin_t, bq, bqr):
                    # out[:, ac, g*512:+512] = cos*W@rhs + sin*Wr@rhs (+bias)
                    for ac in range(NAC):
                        for g in range(n_groups):
                            sl = slice(g * 512, (g + 1) * 512)
                            ps_a = ps3.tile([P, 512], FP32, tag="ps3")
                            ps_b = ps3.tile([P, 512], FP32, tag="ps3")
                            for w, ps, bb in ((wn, ps_a, bq), (wrn, ps_b, bqr)):
                                for dc in range(NAC):
                                    nc.tensor.matmul(
                                        ps[:],
                                        lhsT=w_bf[w][:, dc, ac * P : (ac + 1) * P],
                                        rhs=rhs_tile[:, dc, sl],
                                        start=(dc == 0),
                                        stop=(dc == NAC - 1) and not use_bias,
                                    )
                                if use_bias:
                                    nc.tensor.matmul(
                                        ps[:],
                                        lhsT=bias_bf[:, bb, ac * P : (ac + 1) * P],
                                        rhs=ones_row[:],
                                        start=False,
                                        stop=True,
                                    )
                            tt = tmp_pool.tile([P, 512], BF16, tag="ropetmp")
                            nc.vector.tensor_tensor(tt[:], ps_b[:], sin_t[:, sl], op=ALU.mult)
                            nc.vector.tensor_tensor(
                                out_tile[:, ac, sl], ps_a[:], cos_t[:, sl], op=ALU.mult
                            )
                            nc.vector.tensor_tensor(
                                out_tile[:, ac, sl],
                                out_tile[:, ac, sl],
                                tt[:],
                                op=ALU.add,
                            )

                # V^T projection first: O matmuls of head-pair 0 need it early
                for lc in range(NLC):
                    ps_v = ps3.tile([P, 512], FP32, tag="ps3")
                    for dc in range(NAC):
                        nc.tensor.matmul(
                            ps_v[:],
                            lhsT=ctx_bf[:, dc, lc * P : (lc + 1) * P],
                            rhs=w_bf["wv"][:, dc, :],
                            start=(dc == 0),
                            stop=(dc == NAC - 1) and not use_bias,
                        )
                    if use_bias:
                        nc.tensor.matmul(
                            ps_v[:],
                            lhsT=ones_col[:],
                            rhs=bias_bf[:, 4, :],
                            start=False,
                            stop=True,
                        )
                    nc.vector.tensor_copy(
                        v1[:, lc, :, 0:HEAD_DIM],
                        ps_v[:].rearrange("p (h d) -> p h d", d=HEAD_DIM),
                    )

                proj_pair("wq", "wqr", x_bf, T // 512, q_rope, cos_q, sin_q, 0, 1)
                proj_pair("wk", "wkr", ctx_bf, L // 512, k_rope, cos_k, sin_k, 2, 3)

            # ---- phase 4: attention per head pair ----
            with (
                tc.tile_pool(name="psS", bufs=2, space="PSUM") as psS,
                tc.tile_pool(name="psO", bufs=2, space="PSUM") as psO,
            ):
                for hp in range(NAC):
                    h_a, h_b = 2 * hp, 2 * hp + 1
                    po_a = psO.tile([HEAD_DIM + 1, T], FP32, tag="po")
                    po_b = psO.tile([HEAD_DIM + 1, T], FP32, tag="po")

                    def emit_o(q4, e_a, e_b, h_a=h_a, h_b=h_b, po_a=po_a, po_b=po_b):
                        for tg in range(T // 512):
                            sl = slice(tg * 512, (tg + 1) * 512)
                            for lc4 in range(2):
                                lc = q4 * 2 + lc4
                                nc.tensor.matmul(
                                    po_a[:, sl],
                                    lhsT=v1[:, lc, h_a, :],
                                    rhs=e_a[:, lc4, sl],
                                    start=(lc == 0),
                                    stop=(lc == NLC - 1),
                                )
                                nc.tensor.matmul(
                                    po_b[:, sl],
                                    lhsT=v1[:, lc, h_b, :],
                                    rhs=e_b[:, lc4, sl],
                                    start=(lc == 0),
                                    stop=(lc == NLC - 1),
                                )

                    pending = None
                    for q4 in range(8):
                        e_a = e_pool.tile([P, 2, T], BF16, tag="eA")
                        e_b = e_pool.tile([P, 2, T], BF16, tag="eB")
                        for lc4 in range(2):
                            lc = q4 * 2 + lc4
                            s_a = psS.tile([P, T], FP32, tag="s")
                            s_b = psS.tile([P, T], FP32, tag="s")
                            for rows, s_t in (
                                (slice(0, 64), s_a),
                                (slice(64, 128), s_b),
                            ):
                                for tg in range(T // 512):
                                    sl = slice(tg * 512, (tg + 1) * 512)
                                    nc.tensor.matmul(
                                        s_t[:, sl],
                                        lhsT=k_rope[rows, hp, lc * P : (lc + 1) * P],
                                        rhs=q_rope[rows, hp, sl],
                                        start=True,
                                        stop=True,
                                    )
                            eb = logcm_sb[:, lc : lc + 1] if use_cmask else zero_b[:]
                            nc.scalar.activation(
                                e_a[:, lc4], s_a[:], AF.Exp, bias=eb, scale=inv_scale
                            )
                            nc.scalar.activation(
                                e_b[:, lc4], s_b[:], AF.Exp, bias=eb, scale=inv_scale
                            )
                        if pending is not None:
                            emit_o(*pending)
                        pending = (q4, e_a, e_b)
                    emit_o(*pending)
                    # normalize: O_h / s_h  -> o_norm rows for head h
                    for ps, h in ((po_a, h_a), (po_b, h_b)):
                        rec = nrm_pool.tile([1, T], FP32, tag="rec")
                        nc.vector.reciprocal(rec[:], ps[HEAD_DIM : HEAD_DIM + 1, :])
                        rb = nrm_pool.tile([64, T], FP32, tag="rb")
                        nc.gpsimd.partition_broadcast(rb[:], rec[:], channels=64)
                        r0 = (h % 2) * 64
                        nc.vector.tensor_tensor(
                            o_norm[r0 : r0 + 64, h // 2, :],
                            ps[0:HEAD_DIM, :],
                            rb[:],
                            op=ALU.mult,
                        )

            # ---- phase 5: output projection + mask ----
            with tc.tile_pool(name="ps5", bufs=4, space="PSUM") as ps5:
                for dmc in range(NAC):
                    for tg in range(T // 512):
                        sl = slice(tg * 512, (tg + 1) * 512)
                        po = ps5.tile([P, 512], FP32, tag="ps5")
                        for ac in range(NAC):
                            nc.tensor.matmul(
                                po[:],
                                lhsT=w_bf["wo"][:, ac, dmc * P : (dmc + 1) * P],
                                rhs=o_norm[:, ac, sl],
                                start=(ac == 0),
                                stop=(ac == NAC - 1) and not use_bias,
                            )
                        if use_bias:
                            nc.tensor.matmul(
                                po[:],
                                lhsT=bias_bf[:, 5, dmc * P : (dmc + 1) * P],
                                rhs=ones_row[:],
                                start=False,
                                stop=True,
                            )
                        ot = out_pool.tile([P, 512], FP32, tag="ot")
                        if use_xmask:
                            nc.vector.tensor_tensor(
                                ot[:], po[:], xmask_sb[:, sl], op=ALU.mult
                            )
                        else:
                            nc.vector.tensor_copy(ot[:], po[:])
                        nc.sync.dma_start(out_d[dmc * P : (dmc + 1) * P, sl], ot[:])

    nc.compile()
    return nc


def _rot_rows(w: np.ndarray) -> np.ndarray:
    """Apply the rotate-half permutation R' on the attn-dim axis (rows):
    row (h,j<32) <- -row (h,32+j);  row (h,32+j) <- +row (h,j)."""
    out = np.empty_like(w)
    for h in range(NUM_HEADS):
        blk = w[h * HEAD_DIM : (h + 1) * HEAD_DIM]
        out[h * HEAD_DIM : h * HEAD_DIM + 32] = -blk[32:64]
        out[h * HEAD_DIM + 32 : (h + 1) * HEAD_DIM] = blk[0:32]
    return out


def kernel(
    x,
    context,
    x_mask,
    context_mask,
    Wq_w,
    Wq_b,
    Wk_w,
    Wk_b,
    Wv_w,
    Wv_b,
    Wo_w,
    Wo_b,
    _want_trace=False,
):
    _ensure_ntff_hook()
    x = np.asarray(x, np.float32)
    context = np.asarray(context, np.float32)
    x_mask = np.asarray(x_mask, np.float32)
    context_mask = np.asarray(context_mask, np.float32)
    weights = {
        "wq": np.ascontiguousarray(np.asarray(Wq_w, np.float32).T),
        "wqr": np.ascontiguousarray(_rot_rows(np.asarray(Wq_w, np.float32)).T),
        "wk": np.ascontiguousarray(np.asarray(Wk_w, np.float32).T),
        "wkr": np.ascontiguousarray(_rot_rows(np.asarray(Wk_w, np.float32)).T),
        "wv": np.ascontiguousarray(np.asarray(Wv_w, np.float32).T),
        "wo": np.ascontiguousarray(np.asarray(Wo_w, np.float32).T),
    }
    biases = np.stack(
        [
            np.asarray(Wq_b, np.float32),
            _rot_rows(np.asarray(Wq_b, np.float32)[:, None])[:, 0],
            np.asarray(Wk_b, np.float32),
            _rot_rows(np.asarray(Wk_b, np.float32)[:, None])[:, 0],
            np.asarray(Wv_b, np.float32),
            np.asarray(Wo_b, np.float32),
        ]
    )  # [6, 512]

    use_bias = bool(np.any(biases != 0.0))
    use_cmask = not bool(np.all(context_mask == 1.0))
    use_xmask = not bool(np.all(x_mask == 1.0))

    key = (use_bias, use_cmask, use_xmask)
    if key not in _GRAPH_CACHE:
        _GRAPH_CACHE[key] = _build_graph(*key)
    nc = _GRAPH_CACHE[key]

    len_q = x_mask.sum(axis=(1, 2))  # [B]
    len_k = context_mask.sum(axis=(1, 2))
    theta = (1.0 / (10000.0 ** (np.arange(32, dtype=np.float64) / 32.0))) * ROPE_GAMMA
    theta128 = np.tile(theta, 4)  # row p -> theta_{p%32}

    in_maps = []
    for c in range(N_CORES):
        b, th = c // 2, c % 2
        t0 = th * T
        rope = np.zeros((P, 6), np.float32)
        rope[:, 0] = theta128 / len_q[b]
        rope[:, 1] = theta128 * t0 / len_q[b]
        rope[:, 2] = rope[:, 1] + np.pi / 2
        rope[:, 3] = theta128 / len_k[b]
        rope[:, 4] = 0.0
        rope[:, 5] = np.pi / 2
        m = {
            "x": np.ascontiguousarray(x[b, :, t0 : t0 + T]),
            "ctxT": np.ascontiguousarray(context[b].T),
            "rope": rope,
            **weights,
        }
        if use_bias:
            m["biases"] = biases
        if use_cmask:
            with np.errstate(divide="ignore"):
                lcm = np.log(context_mask[b, 0]).astype(np.float32)  # [L]
            m["logcm"] = np.ascontiguousarray(lcm.reshape(NLC, P).T)
        if use_xmask:
            m["xmaskb"] = np.ascontiguousarray(
                np.broadcast_to(x_mask[b, 0, t0 : t0 + T], (P, T))
            )
        in_maps.append(m)

    res = run_bass_kernel_spmd(
        nc, in_maps, core_ids=list(range(N_CORES)), trace=_want_trace
    )
    out = np.empty((B, D_MODEL, T_FULL), np.float32)
    for c in range(N_CORES):
        b, th = c // 2, c % 2
        out[b, :, th * T : (th + 1) * T] = res.results[c]["out"]
    if _want_trace:
        return out, res
    return out


# revision 17
# speedup vs baseline: 1.3609x; 1.3609x over previous
"""Trainium2 Bass kernel for the AttentionModule problem.

Cross-attention with normalized-position RoPE:
  q = Wq @ x;  k = Wk @ ctx;  v = Wv @ ctx  (per-head RoPE on q, k)
  out = Wo @ (softmax(q^T k / sqrt(512)) @ v), masked.

Sharding: 8 cores = 4 batches x 2 T-halves. Each core computes the full
module for (batch b, query half th) with all heads; host concatenates.
No collectives needed.

Layouts on device (feature-major, partition = feature):
  x_c   [512 dm, 1024 t],  ctxT [512 dc, 2048 l]  (host-transposed)
  Q/K   [a, t|l] via matmul lhsT=W^T chunks; RoPE via twin projection with
        host-rotated weights (Qr = (R'Wq) @ x) and on-device sin/cos tables.
  S_h   [l, t] = K_h^T Q_h (row-paired head pairs on the PE array)
  E     = exp(S/scale + log cmask)  (ACT, no max-subtraction: |logits|<~0.4)
  O_h   [65, t] = [V_h | 1]^T E  -> row 64 holds softmax denominators
  out   [dm, t] = Wo^T (O / s) * xmask
"""

import math
import sys
import types

sys.path.insert(0, "/opt/trn_rl_repo")

import numpy as np

import concourse.bass as bass
import concourse.tile as tile
from concourse import bacc, mybir
from concourse.bass_utils import run_bass_kernel_spmd

# Problem constants (hardcoded per spec; kernel.py must be self-contained)
D_MODEL = 512
D_CONTEXT = 512
NUM_HEADS = 8
ATTN_DIM = 512
HEAD_DIM = 64
ROPE_GAMMA = 10.0
ATTN_SCALE = math.sqrt(ATTN_DIM)
B = 4
T_FULL = 2048
L = 2048
N_CORES = 8
T = T_FULL // 2  # per-core query slice
P = 128
NAC = ATTN_DIM // P  # 4 chunks of 128 on the feature dim
NLC = L // P  # 16 l-chunks
FP32 = mybir.dt.float32
BF16 = mybir.dt.bfloat16
AF = mybir.ActivationFunctionType
ALU = mybir.AluOpType

_GRAPH_CACHE = {}


def _ensure_ntff_hook():
    """antenv.axon_hooks is absent in some images; inject it so trace=True
    can produce exec_time_ns. Harmless if tracing is never requested."""
    if "antenv.axon_hooks" in sys.modules:
        return
    try:
        mod = types.ModuleType("antenv.axon_hooks")
        mod._hook = None
        mod.set_axon_ntff_profile_hook = lambda h: setattr(mod, "_hook", h)
        mod.get_axon_ntff_profile_hook = lambda: mod._hook
        sys.modules["antenv.axon_hooks"] = mod
        from trn_agent_boot.trn_boot import _ntff_profile_via_ctypes

        mod.set_axon_ntff_profile_hook(
            _ntff_profile_via_ctypes("/opt/axon/libaxon_pjrt.so")
        )
    except Exception:
        pass


def _build_graph(use_bias: bool, use_cmask: bool, use_xmask: bool):
    nc = bacc.Bacc("TRN2", target_bir_lowering=False, debug=False, num_devices=N_CORES)

    x_d = nc.dram_tensor("x", [D_MODEL, T], FP32, kind="ExternalInput").ap()
    ctx_d = nc.dram_tensor("ctxT", [D_CONTEXT, L], FP32, kind="ExternalInput").ap()
    w_d = {
        name: nc.dram_tensor(name, [512, 512], FP32, kind="ExternalInput").ap()
        for name in ("wq", "wqr", "wk", "wkr", "wv", "wo")
    }
    rope_d = nc.dram_tensor("rope", [P, 6], FP32, kind="ExternalInput").ap()
    if use_bias:
        bias_d = nc.dram_tensor("biases", [6, 512], FP32, kind="ExternalInput").ap()
    if use_cmask:
        logcm_d = nc.dram_tensor("logcm", [P, NLC], FP32, kind="ExternalInput").ap()
    if use_xmask:
        xmask_d = nc.dram_tensor("xmaskb", [P, T], FP32, kind="ExternalInput").ap()
    out_d = nc.dram_tensor("out", [D_MODEL, T], FP32, kind="ExternalOutput").ap()

    inv_scale = 1.0 / ATTN_SCALE

    with tile.TileContext(nc) as tc:
        with (
            tc.tile_pool(name="const", bufs=1) as const,
            tc.tile_pool(name="big", bufs=1) as big,
            tc.tile_pool(name="stage", bufs=2) as stage,
            tc.tile_pool(name="tmp", bufs=2) as tmp_pool,
            tc.tile_pool(name="nrm", bufs=2) as nrm_pool,
            tc.tile_pool(name="outp", bufs=3) as out_pool,
            tc.tile_pool(name="epool", bufs=2) as e_pool,
        ):
            # ---- constants: rope params, iota, sin/cos tables ----
            rope_sb = const.tile([P, 6], FP32)
            nc.sync.dma_start(rope_sb[:], rope_d[:])
            iota_t = stage.tile([P, L], FP32, tag="stage")
            nc.gpsimd.iota(
                iota_t[:],
                [[1, L]],
                channel_multiplier=0,
                allow_small_or_imprecise_dtypes=True,
            )
            # Sin(x) on ACT needs x in [-pi, pi]: angles are in [0, ~11.6],
            # so shift by -2pi and wrap once (add_range_wrap covers
            # ang in [-pi, 5pi]), then evaluate Sin.
            cos_q = const.tile([P, T], FP32)
            sin_q = const.tile([P, T], FP32)
            cos_k = const.tile([P, L], FP32)
            sin_k = const.tile([P, L], FP32)
            ang = const.tile([P, L], FP32)
            zero_b = const.tile([P, 1], FP32)
            nc.vector.memset(zero_b[:], 0.0)
            two_pi = 2.0 * math.pi
            for table, n, s_ap, b_ap in (
                (cos_q, T, rope_sb[:, 0:1], rope_sb[:, 2:3]),
                (sin_q, T, rope_sb[:, 0:1], rope_sb[:, 1:2]),
                (cos_k, L, rope_sb[:, 3:4], rope_sb[:, 5:6]),
                (sin_k, L, rope_sb[:, 3:4], rope_sb[:, 4:5]),
            ):
                nc.vector.tensor_scalar(
                    ang[:, :n], iota_t[:, :n], s_ap, b_ap, ALU.mult, ALU.add
                )
                nc.vector.add_range_wrap(
                    ang[:, :n], ang[:, :n], shift=-two_pi, bound=math.pi, period=two_pi
                )
                nc.scalar.activation(table[:], ang[:, :n], AF.Sin, bias=zero_b[:])
            if use_cmask:
                logcm_sb = const.tile([P, NLC], FP32)
                nc.sync.dma_start(logcm_sb[:], logcm_d[:])
            if use_xmask:
                xmask_sb = const.tile([P, T], FP32)
                nc.sync.dma_start(xmask_sb[:], xmask_d[:])
            if use_bias:
                bias_st = stage.tile([6, 512], FP32, tag="bstage")
                nc.sync.dma_start(bias_st[:], bias_d[:])
                bias_bf = const.tile([1, 6, 512], BF16)
                nc.vector.tensor_copy(
                    bias_bf[:], bias_st[:].rearrange("b a -> 1 b a")
                )
                ones_row = const.tile([1, 512], BF16)
                nc.vector.memset(ones_row[:], 1.0)
                ones_col = const.tile([1, P], BF16)
                nc.vector.memset(ones_col[:], 1.0)

            # ---- load + cast inputs to bf16 ----
            # order: V-projection inputs (ctx, wv) first so PE work starts
            # as early as possible; x/wq next (Q proj), wo last.
            w_bf = {}

            def load_weight(name):
                st = stage.tile([P, L], FP32, tag="stage")
                stv = st[:].rearrange("p (c a) -> p c a", c=NAC)
                nc.sync.dma_start(stv, w_d[name].rearrange("(c p) a -> p c a", p=P))
                wt = big.tile([P, NAC, 512], BF16, tag=f"w_{name}")
                nc.vector.tensor_copy(wt[:], stv)
                w_bf[name] = wt

            ctx_bf = big.tile([P, NAC, L], BF16)
            for c in range(NAC):
                st = stage.tile([P, L], FP32, tag="stage")
                nc.sync.dma_start(st[:], ctx_d[c * P : (c + 1) * P, :])
                nc.vector.tensor_copy(ctx_bf[:, c], st[:])
            load_weight("wv")
            x_bf = big.tile([P, NAC, T], BF16)
            for c in range(NAC):
                st = stage.tile([P, L], FP32, tag="stage")
                nc.sync.dma_start(st[:, :T], x_d[c * P : (c + 1) * P, :])
                nc.vector.tensor_copy(x_bf[:, c], st[:, :T])
            for name in ("wq", "wqr", "wk", "wkr", "wo"):
                load_weight(name)

            q_rope = big.tile([P, NAC, T], BF16)
            k_rope = big.tile([P, NAC, L], BF16)
            v1 = big.tile([P, NLC, NUM_HEADS, HEAD_DIM + 1], BF16)
            nc.vector.memset(v1[:, :, :, HEAD_DIM : HEAD_DIM + 1], 1.0)
            o_norm = big.tile([P, NAC, T], BF16)

            # ---- phase 3: projections + rope (psum pool scoped) ----
            with tc.tile_pool(name="ps3", bufs=6, space="PSUM") as ps3:

                def proj_pair(wn, wrn, rhs_tile, n_groups, out_tile, cos_t, sin_t, bq, bqr):
                    # out[:, ac, g*512:+512] = cos*W@rhs + sin*Wr@rhs (+bias)
                    for ac in range(NAC):
                        for g in range(n_groups):
                            sl = slice(g * 512, (g + 1) * 512)
                            ps_a = ps3.tile([P, 512], FP32, tag="ps3")
                            ps_b = ps3.tile([P, 512], FP32, tag="ps3")
                            for w, ps, bb in ((wn, ps_a, bq), (wrn, ps_b, bqr)):
                                for dc in range(NAC):
                                    nc.tensor.matmul(
                                        ps[:],
                                        lhsT=w_bf[w][:, dc, ac * P : (ac + 1) * P],
                                        rhs=rhs_tile[:, dc, sl],
                                        start=(dc == 0),
                                        stop=(dc == NAC - 1) and not use_bias,
                                    )
                                if use_bias:
                                    nc.tensor.matmul(
                                        ps[:],
                                        lhsT=bias_bf[:, bb, ac * P : (ac + 1) * P],
                                        rhs=ones_row[:],
                                        start=False,
                                        stop=True,
                                    )
                            tt = tmp_pool.tile([P, 512], BF16, tag="ropetmp")
                            nc.vector.tensor_tensor(tt[:], ps_b[:], sin_t[:, sl], op=ALU.mult)
                            nc.vector.tensor_tensor(
                                out_tile[:, ac, sl], ps_a[:], cos_t[:, sl], op=ALU.mult
                            )
                            nc.vector.tensor_tensor(
                                out_tile[:, ac, sl],
                                out_tile[:, ac, sl],
                                tt[:],
                                op=ALU.add,
                            )

                # V^T projection first: O matmuls of head-pair 0 need it early
                for lc in range(NLC):
                    ps_v = ps3.tile([P, 512], FP32, tag="ps3")
                    for dc in range(NAC):
                        nc.tensor.matmul(
                            ps_v[:],
                            lhsT=ctx_bf[:, dc, lc * P : (lc + 1) * P],
                            rhs=w_bf["wv"][:, dc, :],
                            start=(dc == 0),
                            stop=(dc == NAC - 1) and not use_bias,
                        )
                    if use_bias:
                        nc.tensor.matmul(
                            ps_v[:],
                            lhsT=ones_col[:],
                            rhs=bias_bf[:, 4, :],
                            start=False,
                            stop=True,
                        )
                    nc.vector.tensor_copy(
                        v1[:, lc, :, 0:HEAD_DIM],
                        ps_v[:].rearrange("p (h d) -> p h d", d=HEAD_DIM),
                    )

                proj_pair("wq", "wqr", x_bf, T // 512, q_rope, cos_q, sin_q, 0, 1)
                proj_pair("wk", "wkr", ctx_bf, L // 512, k_rope, cos_k, sin_k, 2, 3)

            # ---- phase 4: attention per head pair ----
            with (
                tc.tile_pool(name="psS", bufs=2, space="PSUM") as psS,
                tc.tile_pool(name="psO", bufs=2, space="PSUM") as psO,
            ):
                for hp in range(NAC):
                    h_a, h_b = 2 * hp, 2 * hp + 1
                    po_a = psO.tile([HEAD_DIM + 1, T], FP32, tag="po")
                    po_b = psO.tile([HEAD_DIM + 1, T], FP32, tag="po")

                    def emit_o(q4, e_a, e_b, h_a=h_a, h_b=h_b, po_a=po_a, po_b=po_b):
                        for tg in range(T // 512):
                            sl = slice(tg * 512, (tg + 1) * 512)
                            for lc4 in range(2):
                                lc = q4 * 2 + lc4
                                nc.tensor.matmul(
                                    po_a[:, sl],
                                    lhsT=v1[:, lc, h_a, :],
                                    rhs=e_a[:, lc4, sl],
                                    start=(lc == 0),
                                    stop=(lc == NLC - 1),
                                )
                                nc.tensor.matmul(
                                    po_b[:, sl],
                                    lhsT=v1[:, lc, h_b, :],
                                    rhs=e_b[:, lc4, sl],
                                    start=(lc == 0),
                                    stop=(lc == NLC - 1),
                                )

                    pending = None
                    for q4 in range(8):
                        e_a = e_pool.tile([P, 2, T], BF16, tag="eA")
                        e_b = e_pool.tile([P, 2, T], BF16, tag="eB")
                        for lc4 in range(2):
                            lc = q4 * 2 + lc4
                            s_a = psS.tile([P, T], FP32, tag="s")
                            s_b = psS.tile([P, T], FP32, tag="s")
                            for rows, s_t in (
                                (slice(0, 64), s_a),
                                (slice(64, 128), s_b),
                            ):
                                for tg in range(T // 512):
                                    sl = slice(tg * 512, (tg + 1) * 512)
                                    nc.tensor.matmul(
                                        s_t[:, sl],
                                        lhsT=k_rope[rows, hp, lc * P : (lc + 1) * P],
                                        rhs=q_rope[rows, hp, sl],
                                        start=True,
                                        stop=True,
                                    )
                            eb = logcm_sb[:, lc : lc + 1] if use_cmask else zero_b[:]
                            nc.scalar.activation(
                                e_a[:, lc4], s_a[:], AF.Exp, bias=eb, scale=inv_scale
                            )
                            nc.scalar.activation(
                                e_b[:, lc4], s_b[:], AF.Exp, bias=eb, scale=inv_scale
                            )
                        if pending is not None:
                            emit_o(*pending)
                        pending = (q4, e_a, e_b)
                    emit_o(*pending)
                    # normalize: O_h / s_h  -> o_norm rows for head h
                    for ps, h in ((po_a, h_a), (po_b, h_b)):
                        rec = nrm_pool.tile([1, T], FP32, tag="rec")
                        nc.vector.reciprocal(rec[:], ps[HEAD_DIM : HEAD_DIM + 1, :])
                        rb = nrm_pool.tile([64, T], FP32, tag="rb")
                        nc.gpsimd.partition_broadcast(rb[:], rec[:], channels=64)
                        r0 = (h % 2) * 64
                        nc.vector.tensor_tensor(
                            o_norm[r0 : r0 + 64, h // 2, :],
                            ps[0:HEAD_DIM, :],
                            rb[:],
                            op=ALU.mult,
                        )

            # ---- phase 5: output projection + mask ----
            with tc.tile_pool(name="ps5", bufs=2, space="PSUM") as ps5:
                for dmc in range(NAC):
                    po = ps5.tile([P, T], FP32, tag="ps5")
                    for tg in range(T // 512):
                        sl = slice(tg * 512, (tg + 1) * 512)
                        for ac in range(NAC):
                            nc.tensor.matmul(
                                po[:, sl],
                                lhsT=w_bf["wo"][:, ac, dmc * P : (dmc + 1) * P],
                                rhs=o_norm[:, ac, sl],
                                start=(ac == 0),
                                stop=(ac == NAC - 1) and not use_bias,
                            )
                        if use_bias:
                            nc.tensor.matmul(
                                po[:, sl],
                                lhsT=bias_bf[:, 5, dmc * P : (dmc + 1) * P],
                                rhs=ones_row[:],
                                start=False,
                                stop=True,
                            )
                    ot = out_pool.tile([P, T], FP32, tag="ot")
                    if use_xmask:
                        nc.vector.tensor_tensor(ot[:], po[:], xmask_sb[:], op=ALU.mult)
                    else:
                        nc.vector.tensor_copy(ot[:], po[:])
                    nc.sync.dma_start(out_d[dmc * P : (dmc + 1) * P, :], ot[:])

    nc.compile()
    return nc


def _rot_rows(w: np.ndarray) -> np.ndarray:
    """Apply the rotate-half permutation R' on the attn-dim axis (rows):
    row (h,j<32) <- -row (h,32+j);  row (h,32+j) <- +row (h,j)."""
    out = np.empty_like(w)
    for h in range(NUM_HEADS):
        blk = w[h * HEAD_DIM : (h + 1) * HEAD_DIM]
        out[h * HEAD_DIM : h * HEAD_DIM + 32] = -blk[32:64]
        out[h * HEAD_DIM + 32 : (h + 1) * HEAD_DIM] = blk[0:32]
    return out


def kernel(
    x,
    context,
    x_mask,
    context_mask,
    Wq_w,
    Wq_b,
    Wk_w,
    Wk_b,
    Wv_w,
    Wv_b,
    Wo_w,
    Wo_b,
    _want_trace=False,
):
    _ensure_ntff_hook()
    x = np.asarray(x, np.float32)
    context = np.asarray(context, np.float32)
    x_mask = np.asarray(x_mask, np.float32)
    context_mask = np.asarray(context_mask, np.float32)
    weights = {
        "wq": np.ascontiguousarray(np.asarray(Wq_w, np.float32).T),
        "wqr": np.ascontiguousarray(_rot_rows(np.asarray(Wq_w, np.float32)).T),
        "wk": np.ascontiguousarray(np.asarray(Wk_w, np.float32).T),
        "wkr": np.ascontiguousarray(_rot_rows(np.asarray(Wk_w, np.float32)).T),
        "wv": np.ascontiguousarray(np.asarray(Wv_w, np.float32).T),
        "wo": np.ascontiguousarray(np.asarray(Wo_w, np.float32).T),
    }
    biases = np.stack(
        [
            np.asarray(Wq_b, np.float32),
            _rot_rows(np.asarray(Wq_b, np.float32)[:, None])[:, 0],
            np.asarray(Wk_b, np.float32),
            _rot_rows(np.asarray(Wk_b, np.float32)[:, None])[:, 0],
            np.asarray(Wv_b, np.float32),
            np.asarray(Wo_b, np.float32),
        ]
    )  # [6, 512]

    use_bias = bool(np.any(biases != 0.0))
    use_cmask = not bool(np.all(context_mask == 1.0))
    use_xmask = not bool(np.all(x_mask == 1.0))

    key = (use_bias, use_cmask, use_xmask)
    if key not in _GRAPH_CACHE:
        _GRAPH_CACHE[key] = _build_graph(*key)
    nc = _GRAPH_CACHE[key]

    len_q = x_mask.sum(axis=(1, 2))  # [B]
    len_k = context_mask.sum(axis=(1, 2))
    theta = (1.0 / (10000.0 ** (np.arange(32, dtype=np.float64) / 32.0))) * ROPE_GAMMA
    theta128 = np.tile(theta, 4)  # row p -> theta_{p%32}

    in_maps = []
    for c in range(N_CORES):
        b, th = c // 2, c % 2
        t0 = th * T
        rope = np.zeros((P, 6), np.float32)
        rope[:, 0] = theta128 / len_q[b]
        rope[:, 1] = theta128 * t0 / len_q[b]
        rope[:, 2] = rope[:, 1] + np.pi / 2
        rope[:, 3] = theta128 / len_k[b]
        rope[:, 4] = 0.0
        rope[:, 5] = np.pi / 2
        m = {
            "x": np.ascontiguousarray(x[b, :, t0 : t0 + T]),
            "ctxT": np.ascontiguousarray(context[b].T),
            "rope": rope,
            **weights,
        }
        if use_bias:
            m["biases"] = biases
        if use_cmask:
            with np.errstate(divide="ignore"):
                lcm = np.log(context_mask[b, 0]).astype(np.float32)  # [L]
            m["logcm"] = np.ascontiguousarray(lcm.reshape(NLC, P).T)
        if use_xmask:
            m["xmaskb"] = np.ascontiguousarray(
                np.broadcast_to(x_mask[b, 0, t0 : t0 + T], (P, T))
            )
        in_maps.append(m)

    res = run_bass_kernel_spmd(
        nc, in_maps, core_ids=list(range(N_CORES)), trace=_want_trace
    )
    out = np.empty((B, D_MODEL, T_FULL), np.float32)
    for c in range(N_CORES):
        b, th = c // 2, c % 2
        out[b, :, th * T : (th + 1) * T] = res.results[c]["out"]
    if _want_trace:
        return out, res
    return out
